# revision 34
# baseline (speedup 1.0000x reference)
"""PeriodicSetTransformer Trainium2 kernel.

Strategy: data-parallel over batch B=32 across 8 cores (4 crystals/core),
weights replicated. Host folds the embedding (AtomFeaturizer + quadratic
DistanceExpansion) into x_init and folds ln_g / attention scale / the
MultiheadAttention in-projection into per-layer GEMM weights WQ/WK/WV.
Device runs the 3 encoder layers + weighted pool + final LN + head.
"""

import numpy as np
from contextlib import ExitStack

import concourse.bass as bass
import concourse.bacc as bacc
import concourse.tile as tile
from concourse import mybir
from concourse.bass_utils import run_bass_kernel_spmd
from concourse.masks import make_identity

FP = mybir.dt.float32
AOP = mybir.AluOpType
AFT = mybir.ActivationFunctionType
AXX = mybir.AxisListType.X

B, N, E, H, L, EH = 32, 512, 128, 4, 3, 512
S, F, TAB = 10, 101, 100
NCORES = 8
CPC = B // NCORES          # crystals per core = 4
P = 128
NCH = N // P               # 4 row chunks per crystal
HD = EH // H               # 128 head dim


# ---------------------------------------------------------------- host side

def _prep(inputs):
    f32, f64 = np.float32, np.float64
    g = lambda k: np.asarray(inputs[k])

    str_fea = g('str_fea').astype(f32)
    comp_fea = g('comp_fea').astype(np.int64)
    ln_g, ln_b = g('ln_g'), g('ln_b')
    ln2_g, ln2_b = g('ln2_g'), g('ln2_b')
    assert np.all(ln_g == 1.0) and np.all(ln_b == 0.0), "nontrivial ln affine"
    assert np.all(ln2_g == 1.0) and np.all(ln2_b == 0.0), "nontrivial ln2 affine"
    for k in ('bq', 'bk', 'bv', 'inproj_b', 'out_b', 'ffn_b'):
        assert not np.any(g(k)), f"nonzero bias {k}"

    # x_init = af_table[comp]@comp_w + de@emb_w (+biases), with the quadratic
    # distance expansion folded:  sum_i (a_i - d)^2 W[f,i] = d^2 W0 + d W1 + W2
    a = 1.0 + np.arange(S, dtype=f64) / S
    W = g('emb_w').astype(f64).reshape(F - 1, S, E)
    W0 = W.sum(1)
    W1 = -2.0 * np.einsum('s,fse->fe', a, W)
    W2 = np.einsum('s,fse->e', a * a, W) + g('emb_b').astype(f64)
    table2 = g('af_table').astype(f64) @ g('comp_w').astype(f64) \
        + g('comp_b').astype(f64) + W2
    d = str_fea[:, :, 1:].astype(f64)
    x_init = (table2[comp_fea] + (d * d) @ W0 + d @ W1).astype(f32)  # [B,N,E]
    w_row = np.ascontiguousarray(str_fea[:, :, 0])                   # [B,N]
    has_zero = bool((w_row == 0.0).any())
    assert not has_zero, "kernel assumes no zero site weights (mask path removed)"

    # fold in-projection + scale into q/k weights
    scale = 1.0 / np.sqrt(HD)
    wq, wk, wv = g('wq').astype(f64), g('wk').astype(f64), g('wv').astype(f64)
    ipw = g('inproj_w').astype(f64)
    WQ = np.stack([(wq[l] @ ipw[l][:, :EH]) * scale for l in range(L)])
    WK = np.stack([wk[l] @ ipw[l][:, EH:2 * EH] for l in range(L)])
    WV = wv
    wqkv = np.stack([WQ, WK, WV], 1).astype(f32)                     # [L,3,E,EH]
    wqkv_pack = np.ascontiguousarray(wqkv.transpose(2, 0, 1, 3))     # [E,L,3,EH]

    out_w = g('out_w').astype(f32)                                   # [L,EH,E]
    ow_pack = np.ascontiguousarray(
        out_w.reshape(L, NCH, P, E).transpose(2, 0, 1, 3)).reshape(P, L, NCH * E)
    fw_pack = np.ascontiguousarray(g('ffn_w').astype(f32).transpose(1, 0, 2))  # [E,L,E]
    final_w = g('final_w').astype(f32)                               # [E,1]
    final_b = g('final_b').astype(f32)                               # [1]
    fwb = np.ascontiguousarray(
        np.tile(final_w[:, 0][None, None, :], (1, CPC, 1)))          # [1,CPC,E]

    shared = {'wqkv': wqkv_pack, 'ow': ow_pack, 'fw': fw_pack, 'fwb': fwb}
    percore = []
    for c in range(NCORES):
        b0 = c * CPC
        xi = x_init[b0:b0 + CPC]                                     # [4,512,128]
        xi_pack = np.ascontiguousarray(
            xi.reshape(CPC, NCH, P, E).transpose(2, 0, 1, 3))        # [128,4,4,128]
        wr = np.ascontiguousarray(w_row[b0:b0 + CPC])                # [4,512]
        wcols = np.ascontiguousarray(
            wr.reshape(CPC, NCH, P).transpose(2, 0, 1)).reshape(P, CPC * NCH)
        percore.append({'xinit': xi_pack, 'wrow': wr, 'wcols': wcols,
                        'wflat': wr.reshape(1, CPC * N)})

    aux = {'x_init': x_init, 'w_row': w_row, 'WQ': WQ.astype(f32),
           'WK': WK.astype(f32), 'WV': WV.astype(f32), 'OW': out_w,
           'FW': g('ffn_w').astype(f32), 'final_w': final_w, 'final_b': final_b,
           'has_zero': has_zero}
    return shared, percore, aux


# ---------------------------------------------------------------- device side

def _fused_ln(nc, tp, sp, out3, in3):
    """LayerNorm along the last (free) dim of a [P, G, D] view. eps=1e-5."""
    Pd, G, D = in3.shape
    sm = sp.tile([Pd, G], FP, name="ln_sm")
    nc.vector.reduce_sum(sm[:], in3, axis=AXX)
    smb = sm[:].unsqueeze(-1).broadcast_to([Pd, G, D])
    # out = x - mean = (sm * -1/D) + x
    nc.vector.scalar_tensor_tensor(
        out=out3, in0=smb, scalar=-1.0 / D, in1=in3, op0=AOP.mult, op1=AOP.add)
    sq = tp.tile([Pd, G, D], FP, name="ln_sq")
    nc.vector.tensor_tensor(sq[:], out3, out3, op=AOP.mult)
    s2 = sp.tile([Pd, G], FP, name="ln_s2")
    nc.vector.reduce_sum(s2[:], sq[:], axis=AXX)
    # rstd = (s2/D + eps)^-1/2 = exp(-0.5*ln(.)); ACT set has only exp/ln/copy
    lnv = sp.tile([Pd, G], FP, name="ln_lnv")
    nc.scalar.activation(lnv[:], s2[:], func=AFT.Ln, bias=1e-5, scale=1.0 / D)
    rstd = sp.tile([Pd, G], FP, name="ln_rstd")
    nc.scalar.activation(rstd[:], lnv[:], func=AFT.Exp, scale=-0.5)
    rb = rstd[:].unsqueeze(-1).broadcast_to([Pd, G, D])
    nc.vector.tensor_tensor(out3, out3, rb, op=AOP.mult)


def _build(nc):
    xinit_d = nc.dram_tensor('xinit', [P, CPC, NCH, E], FP, kind='ExternalInput')
    wflat_d = nc.dram_tensor('wflat', [1, CPC * N], FP, kind='ExternalInput')
    wcols_d = nc.dram_tensor('wcols', [P, CPC * NCH], FP, kind='ExternalInput')
    wqkv_d = nc.dram_tensor('wqkv', [P, L, 3, EH], FP, kind='ExternalInput')
    ow_d = nc.dram_tensor('ow', [P, L, NCH * E], FP, kind='ExternalInput')
    fw_d = nc.dram_tensor('fw', [P, L, E], FP, kind='ExternalInput')
    fwb_d = nc.dram_tensor('fwb', [1, CPC, E], FP, kind='ExternalInput')
    out_d = nc.dram_tensor('out', [1, CPC], FP, kind='ExternalOutput')

    eps_t = nc.alloc_sbuf_tensor('const-float32-eps', [P, 1], FP)
    nc.gpsimd.memset(eps_t.ap(), 1e-5)
    nc.const_aps.aps[(FP, 1e-5)] = eps_t.ap()
    nc.all_engine_barrier()

    with tile.TileContext(nc) as tc, ExitStack() as ctx:
        ko = ctx.enter_context(tc.tile_pool(name="ko", bufs=1))
        cp = ctx.enter_context(tc.tile_pool(name="cp", bufs=2))
        tp = ctx.enter_context(tc.tile_pool(name="tp", bufs=2))
        ep = ctx.enter_context(tc.tile_pool(name="ep", bufs=3))
        sp = ctx.enter_context(tc.tile_pool(name="sp", bufs=4))
        ppb = ctx.enter_context(tc.tile_pool(name="ppb", bufs=3, space="PSUM"))
        pps = ctx.enter_context(tc.tile_pool(name="pps", bufs=2, space="PSUM"))
        ppp = ctx.enter_context(tc.tile_pool(name="ppp", bufs=2, space="PSUM"))

        ident = ko.tile([P, P], FP, name="ident")
        make_identity(nc, ident)
        wqkv_sb = ko.tile([P, L, 3, EH], FP, name="wqkv_sb")
        nc.sync.dma_start(out=wqkv_sb, in_=wqkv_d[:])
        ow_sb = ko.tile([P, L, NCH * E], FP, name="ow_sb")
        nc.sync.dma_start(out=ow_sb, in_=ow_d[:])
        fw_sb = ko.tile([P, L, E], FP, name="fw_sb")
        nc.sync.dma_start(out=fw_sb, in_=fw_d[:])
        fwb_sb = ko.tile([1, CPC, E], FP, name="fwb_sb")
        nc.sync.dma_start(out=fwb_sb, in_=fwb_d[:])
        wcols_sb = ko.tile([P, CPC * NCH], FP, name="wcols_sb")
        nc.sync.dma_start(out=wcols_sb, in_=wcols_d[:])
        xinit_all = ko.tile([P, CPC, NCH, E], FP, name="xinit_all")
        nc.sync.dma_start(out=xinit_all, in_=xinit_d[:])
        # broadcast w over partitions via ones-matmul (Bacc splits the waits)
        wflat_sb = ko.tile([1, CPC * N], FP, name="wflat_sb")
        nc.sync.dma_start(out=wflat_sb, in_=wflat_d[:])
        ones1 = ko.tile([1, P], FP, name="ones1")
        nc.vector.memset(ones1, 1.0)
        wb_all = ko.tile([P, CPC, N], FP, name="wb_all")
        for r in range(CPC):
            psw = ppb.tile([P, N], FP, name="psb")
            nc.tensor.matmul(psw, ones1, wflat_sb[0:1, r * N:(r + 1) * N],
                             start=True, stop=True)
            nc.vector.tensor_copy(wb_all[:, r], psw)
        pooled_all = ko.tile([1, CPC, E], FP, name="pooled_all")

        for r in range(CPC):
            xinit = xinit_all[:, r]
            wb = wb_all[:, r]
            x_tile = cp.tile([P, NCH, E], FP, name="x_tile")
            xcur = xinit

            for l in range(L):
                xn = cp.tile([P, NCH, E], FP, name="xn")
                _fused_ln(nc, tp, sp, xn[:], xcur[:])
                xnT = cp.tile([P, NCH, P], FP, name="xnT")
                pt = pps.tile([P, NCH, P], FP, name="pst")
                for c in range(NCH):
                    nc.tensor.transpose(pt[:, c], xn[:, c], ident)
                nc.vector.tensor_copy(xnT[:], pt[:])
                qT = cp.tile([P, H, N], FP, name="qT")
                kT = cp.tile([P, H, N], FP, name="kT")
                for h in range(H):
                    psq = ppb.tile([P, N], FP, name="psb")
                    nc.tensor.matmul(
                        psq, wqkv_sb[:, l, 0, h * HD:(h + 1) * HD], xnT,
                        start=True, stop=True)
                    nc.vector.tensor_copy(qT[:, h], psq)
                    psk = ppb.tile([P, N], FP, name="psb")
                    nc.tensor.matmul(
                        psk, wqkv_sb[:, l, 1, h * HD:(h + 1) * HD], xnT,
                        start=True, stop=True)
                    nc.vector.tensor_copy(kT[:, h], psk)
                vin = cp.tile([P, NCH, EH], FP, name="vin")
                for jc in range(NCH):
                    psv = ppb.tile([P, EH], FP, name="psb")
                    nc.tensor.matmul(
                        psv, xnT[:, jc], wqkv_sb[:, l, 2],
                        start=True, stop=True)
                    nc.scalar.activation(vin[:, jc], psv, func=AFT.Copy)

                awT = cp.tile([P, NCH, NCH, P], FP, name="awT")  # [j', jc, ic, i']
                for ic in range(NCH):
                    acc = tp.tile([P, N], FP, name="acc")
                    for h in range(H):
                        psl = ppb.tile([P, N], FP, name="psb")
                        nc.tensor.matmul(
                            psl, qT[:, h, ic * P:(ic + 1) * P], kT[:, h],
                            start=True, stop=True)
                        eh = ep.tile([P, N], FP, name="eh")
                        sh = sp.tile([P, 1], FP, name="sh")
                        nc.scalar.activation(eh, psl, func=AFT.Exp, accum_out=sh)
                        rh = sp.tile([P, 1], FP, name="rh")
                        nc.vector.reciprocal(rh, sh)
                        if h == 0:
                            nc.vector.tensor_scalar(
                                out=acc, in0=eh, scalar1=rh, scalar2=None, op0=AOP.mult)
                        else:
                            nc.vector.scalar_tensor_tensor(
                                out=acc, in0=eh, scalar=rh, in1=acc,
                                op0=AOP.mult, op1=AOP.add)
                    taw = tp.tile([P, N], FP, name="taw")
                    zz = sp.tile([P, 1], FP, name="zz")
                    nc.vector.scalar_tensor_tensor(
                        out=taw, in0=acc, scalar=1.0, in1=wb,
                        op0=AOP.mult, op1=AOP.mult, accum_out=zz)
                    rz = sp.tile([P, 1], FP, name="rz")
                    nc.vector.reciprocal(rz, zz)
                    nc.vector.tensor_scalar(
                        out=taw, in0=taw, scalar1=rz, scalar2=None, op0=AOP.mult)
                    pta = pps.tile([P, NCH, P], FP, name="pst")
                    for jc in range(NCH):
                        nc.tensor.transpose(
                            pta[:, jc], taw[:, jc * P:(jc + 1) * P], ident)
                    nc.vector.tensor_copy(awT[:, :, ic, :], pta[:])

                attT = cp.tile([P, NCH, N], FP, name="attT")  # [e'', ec, i]
                for ec in range(NCH):
                    psa = ppb.tile([P, NCH, P], FP, name="psb")
                    for jc in range(NCH):
                        nc.tensor.matmul(
                            psa, vin[:, jc, ec * P:(ec + 1) * P], awT[:, jc],
                            start=(jc == 0), stop=(jc == NCH - 1))
                    nc.scalar.activation(attT[:, ec], psa, func=AFT.Copy)

                ps1 = ppb.tile([P, NCH, E], FP, name="psb")
                for ic in range(NCH):
                    for ec in range(NCH):
                        nc.tensor.matmul(
                            ps1[:, ic], attT[:, ec, ic * P:(ic + 1) * P],
                            ow_sb[:, l, ec * E:(ec + 1) * E],
                            start=(ec == 0), stop=(ec == NCH - 1))
                o1 = cp.tile([P, NCH, E], FP, name="o1")
                nc.vector.tensor_tensor(o1[:], ps1[:], xcur[:], op=AOP.add)

                _fused_ln(nc, tp, sp, xn[:], o1[:])
                xnT2 = cp.tile([P, NCH, P], FP, name="xnT")
                pt2 = pps.tile([P, NCH, P], FP, name="pst")
                for c in range(NCH):
                    nc.tensor.transpose(pt2[:, c], xn[:, c], ident)
                nc.vector.tensor_copy(xnT2[:], pt2[:])
                psf = ppb.tile([P, NCH, E], FP, name="psb")
                for ic in range(NCH):
                    nc.tensor.matmul(
                        psf[:, ic], xnT2[:, ic], fw_sb[:, l],
                        start=True, stop=True)
                o2 = cp.tile([P, NCH, E], FP, name="o2")
                nc.scalar.activation(o2[:], psf[:], func=AFT.Exp)
                nc.scalar.activation(o2[:], o2[:], func=AFT.Ln, bias=1.0)
                nc.vector.tensor_tensor(o2[:], o2[:], o1[:], op=AOP.add)
                _fused_ln(nc, tp, sp, x_tile[:], o2[:])
                xcur = x_tile

            yv = cp.tile([P, NCH, E], FP, name="yv")
            nc.vector.tensor_tensor(yv[:], xcur[:], xinit[:], op=AOP.add)
            psp = ppp.tile([1, E], FP, name="psp")
            for c in range(NCH):
                nc.tensor.matmul(
                    psp, wcols_sb[:, r * NCH + c:r * NCH + c + 1], yv[:, c],
                    start=(c == 0), stop=(c == NCH - 1))
            nc.vector.tensor_copy(pooled_all[0:1, r], psp)

        pln = ko.tile([1, CPC, E], FP, name="pln")
        _fused_ln(nc, tp, sp, pln[:], pooled_all[:])
        hm = ko.tile([1, CPC, E], FP, name="hm")
        res = ko.tile([1, CPC], FP, name="res")
        nc.vector.tensor_tensor(hm[:], pln[:], fwb_sb[:], op=AOP.mult)
        nc.vector.reduce_sum(res[:], hm[:], axis=AXX)
        nc.sync.dma_start(out=out_d[:], in_=res[:])


# ---------------------------------------------------------------- entry points

def _run(inputs, trace=False):
    shared, percore, aux = _prep(inputs)
    nc = bacc.Bacc()
    _build(nc)
    nc.finalize()  # Bacc.compile(): splits multi-waits into event semaphores
    in_maps = [dict(shared, **percore[c]) for c in range(NCORES)]
    res = run_bass_kernel_spmd(nc, in_maps, core_ids=list(range(NCORES)), trace=trace)
    out = np.concatenate(
        [np.asarray(r['out']).reshape(CPC, 1) for r in res.results],
        axis=0).astype(np.float32)
    out = out + aux['final_b'][None, :]
    return out, res


def kernel(**inputs):
    return _run(inputs)[0]
